# revision 1
# baseline (speedup 1.0000x reference)
"""Trainium2 Bass kernel for nn_ODE_71743133713072.

Semantics (unrolled from the reference lax.scan):
  out[:, 0]   = lat[:, 0]
  out[:, t+1] = lat[:, t] + dt_eff[t] * f(lat[:, t])   for t = 0..99
                (dt_eff[1] = 0 reproduces the scan's zero-length first gap)
  y = out[:, 100]
  out[:, k+1] = y = y + h * f(y)                        for k = 100..118
where f is the D->U->U->D tanh MLP and all nonzero dt equal h = ts[1]-ts[0]
(linspace; per-step fp32 diffs differ from h by <=1 ulp, far below the bf16
matmul noise floor, so h is folded into W3/b3 on the host).

Sharding: batch 1024 over 8 cores (128 rows/core, exactly the partition
width). Matmuls in bf16 with fp32 PSUM accumulation. Layers 1-2 run
feature-on-partition (weights stationary); layer 3 swaps roles (activations
stationary, h*W3 moving) so its output lands in natural row layout and the
Euler update is a single PSUM+SBUF add. b3*h enters layer 3's accumulation
group via a K=1 ones-row matmul.
"""

import os
import sys
from contextlib import ExitStack

import numpy as np

for _p in ("/opt/trn_rl_repo", "/root/.axon_site/_ro/trn_rl_repo"):
    if os.path.isdir(_p) and _p not in sys.path:
        sys.path.append(_p)

import ml_dtypes  # noqa: E402

B, T_OBS, KPRED, D = 1024, 100, 20, 256
T = T_OBS + KPRED          # 120
NCORES = 8
PB = B // NCORES           # 128 rows per core
P = 128
G = 4                      # time steps per compute group
NG = T_OBS // G            # 25 groups


def _emit(ctx, tc, lat, w3hd, w8d, bpk, br8p, id8d, id32d, ones8d, out, h):
    import concourse.mybir as mybir

    nc = tc.nc
    F32 = mybir.dt.float32
    BF16 = mybir.dt.bfloat16
    FP8 = mybir.dt.float8e4
    AF = mybir.ActivationFunctionType
    ALU = mybir.AluOpType
    DR = mybir.MatmulPerfMode.DoubleRow

    const = ctx.enter_context(tc.tile_pool(name="const", bufs=1))
    w3sb = const.tile([P, 2, D], BF16, tag="w3")
    for kc in range(2):
        nc.sync.dma_start(w3sb[:, kc, :], w3hd[kc * P:(kc + 1) * P, :])
    bsb = const.tile([P, 6], F32, tag="bias")
    nc.sync.dma_start(bsb[:], bpk[:])
    ones8 = const.tile([1, P], FP8, tag="ones8")
    nc.sync.dma_start(ones8[:], ones8d[:])
    # fp8 weights (x8-scaled): [P, 3(w), 2(kc), D]
    w8sb = const.tile([P, 3, 2, D], FP8, tag="w8")
    for wi in range(3):
        for kc in range(2):
            nc.sync.dma_start(w8sb[:, wi, kc, :], w8d[wi, kc * P:(kc + 1) * P, :])
    br8sb = const.tile([1, 2 * D], FP8, tag="br8")
    nc.sync.dma_start(br8sb[:], br8p[:])
    id16 = const.tile([P, P], BF16, tag="id16")
    nc.sync.dma_start(id16[:], id8d[:])
    id32 = const.tile([P, P], F32, tag="id32")
    nc.sync.dma_start(id32[:], id32d[:])

    b1ap = [bsb[:, 0:1], bsb[:, 1:2]]
    b2ap = [bsb[:, 2:3], bsb[:, 3:4]]
    b3hap = [bsb[:, 4:5], bsb[:, 5:6]]

    x32p = ctx.enter_context(tc.tile_pool(name="x32", bufs=4))
    x8p = ctx.enter_context(tc.tile_pool(name="x8", bufs=4))
    xtsbp = ctx.enter_context(tc.tile_pool(name="xtsb", bufs=3))
    hsbp = ctx.enter_context(tc.tile_pool(name="hsb", bufs=4))
    outp = ctx.enter_context(tc.tile_pool(name="outsb", bufs=4))
    chsb = ctx.enter_context(tc.tile_pool(name="chsb", bufs=3))

    xtps = ctx.enter_context(tc.tile_pool(name="xtps", bufs=1, space="PSUM"))
    mmps = ctx.enter_context(tc.tile_pool(name="mmps", bufs=2, space="PSUM"))
    fnps = ctx.enter_context(tc.tile_pool(name="fnps", bufs=2, space="PSUM"))
    chps = ctx.enter_context(tc.tile_pool(name="chps", bufs=1, space="PSUM"))

    h8 = float(h / 8.0)

    def stage_load(g):
        """load + cast + transpose + evac for one group; returns tiles."""
        t0 = g * G
        x32 = x32p.tile([P, G, D], F32, tag="x32")
        nc.sync.dma_start(x32[:], lat[:, t0:t0 + G, :])
        x16 = x8p.tile([P, G, D], BF16, tag="x16")
        nc.gpsimd.tensor_copy(x16[:], x32[:])
        xt = xtps.tile([P, 2, G * P], BF16, tag="xt")
        for tt in range(G):
            for dc in range(2):
                nc.tensor.transpose(
                    xt[:, dc, tt * P:(tt + 1) * P],
                    x16[:, tt, dc * P:(dc + 1) * P], id16[:])
        xts = xtsbp.tile([P, 2, G * P], FP8, tag="xts")
        for dc in range(2):
            nc.vector.tensor_copy(xts[:, dc, :], xt[:, dc, :])
        return x32, xts

    def stage_mlp(pair):
        """L1/L2 for a pair of groups with shared weight loads."""
        h1s = {}
        mm = {}
        for g, (x32, xts) in pair.items():
            mm[g] = mmps.tile([P, 2, G * P], F32, tag="mm", name="mm")
        for mc in range(2):
            for g in pair:
                nc.tensor.matmul(mm[g][:, mc, :],
                                 w8sb[:, 0, :, mc * P:(mc + 1) * P],
                                 pair[g][1][:], start=True, stop=True,
                                 perf_mode=DR)
        for g in pair:
            t = hsbp.tile([P, 2, G * P], FP8, tag="h1")
            for mc in range(2):
                nc.scalar.activation(t[:, mc, :], mm[g][:, mc, :], AF.Tanh,
                                     bias=b1ap[mc], scale=0.125)
            h1s[g] = t
        mm2 = {}
        for g in pair:
            mm2[g] = mmps.tile([P, 2, G * P], F32, tag="mm", name="mm2")
        for mc in range(2):
            for g in pair:
                nc.tensor.matmul(mm2[g][:, mc, :],
                                 w8sb[:, 1, :, mc * P:(mc + 1) * P],
                                 h1s[g][:], start=True, stop=True,
                                 perf_mode=DR)
        h2s = {}
        for g in pair:
            t = hsbp.tile([P, 2, G * P], FP8, tag="h2")
            for mc in range(2):
                nc.scalar.activation(t[:, mc, :], mm2[g][:, mc, :], AF.Tanh,
                                     bias=b2ap[mc], scale=0.125)
            h2s[g] = t
        return h2s

    def stage_out(g, x32, h2s_g):
        """L3 (role-swapped, fp8 DR) + Euler add + store for one group."""
        t0 = g * G
        o32 = outp.tile([P, G, D], F32, tag="o32")
        for half in range(2):
            fn = fnps.tile([P, 2, D], F32, tag="fn")
            # seed each subtile with 8*b3 broadcast (K=1 ones row)
            for i, tt in enumerate((2 * half, 2 * half + 1)):
                nc.tensor.matmul(fn[:, i, :], ones8[:], br8sb[:, 0:D],
                                 start=True, stop=False)
                nc.tensor.matmul(fn[:, i, :],
                                 h2s_g[:, :, tt * P:(tt + 1) * P],
                                 w8sb[:, 2, :, :],
                                 start=False, stop=True, perf_mode=DR)
            if g == 0 and half == 0:
                # t=0: normal Euler step; t=1: dt=0 -> out[:,2] = lat[:,1]
                nc.vector.scalar_tensor_tensor(
                    o32[:, 0, :], fn[:, 0, :], h8, x32[:, 0, :],
                    ALU.mult, ALU.add)
                nc.vector.tensor_copy(o32[:, 1, :], x32[:, 1, :])
            else:
                nc.vector.scalar_tensor_tensor(
                    o32[:, 2 * half:2 * half + 2, :].rearrange("p a b -> p (a b)"),
                    fn.rearrange("p a b -> p (a b)"), h8,
                    x32[:, 2 * half:2 * half + 2, :].rearrange("p a b -> p (a b)"),
                    ALU.mult, ALU.add)
        nc.sync.dma_start(out[:, t0 + 1:t0 + G + 1, :], o32[:])
        return o32

    def do_pair(ga, gb):
        pair = {}
        for g in (ga, gb):
            if g is not None:
                pair[g] = stage_load(g)
        h2s = stage_mlp(pair)
        outs = {}
        for g in pair:
            outs[g] = stage_out(g, pair[g][0], h2s[g])
        return outs

    def chain(o32_24):
        # y0 = out[:, 100] = o32_24[:, 3, :]; chain state transposed fp32.
        y0p = chps.tile([P, 2, P], F32, tag="ch")
        for dc in range(2):
            nc.tensor.transpose(y0p[:, dc, :],
                                o32_24[:, G - 1, dc * P:(dc + 1) * P], id32[:])
        yt = chsb.tile([P, 2, P], F32, tag="yt")
        nc.vector.tensor_copy(yt[:], y0p[:])

        for k in range(T_OBS, T - 1):
            y8 = chsb.tile([P, 2, P], FP8, tag="y8")
            nc.vector.tensor_copy(y8[:], yt[:])
            c1 = chps.tile([P, 2, P], F32, tag="ch")
            for mc in range(2):
                nc.tensor.matmul(c1[:, mc, :],
                                 w8sb[:, 0, :, mc * P:(mc + 1) * P],
                                 y8[:], start=True, stop=True, perf_mode=DR)
            c1s = chsb.tile([P, 2, P], FP8, tag="c1s")
            for mc in range(2):
                nc.scalar.activation(c1s[:, mc, :], c1[:, mc, :], AF.Tanh,
                                     bias=b1ap[mc], scale=0.125)
            c2 = chps.tile([P, 2, P], F32, tag="ch")
            for mc in range(2):
                nc.tensor.matmul(c2[:, mc, :],
                                 w8sb[:, 1, :, mc * P:(mc + 1) * P],
                                 c1s[:], start=True, stop=True, perf_mode=DR)
            c2s = chsb.tile([P, 2, P], BF16, tag="c2s")
            for mc in range(2):
                nc.scalar.activation(c2s[:, mc, :], c2[:, mc, :], AF.Tanh,
                                     bias=b2ap[mc], scale=0.125)
            # L3 in bf16 (w3sb = h*W3); b3*h joins in the update op below.
            c3 = chps.tile([P, 2, P], F32, tag="ch")
            for mc in range(2):
                for kc in range(2):
                    nc.tensor.matmul(c3[:, mc, :],
                                     w3sb[:, kc, mc * P:(mc + 1) * P],
                                     c2s[:, kc, :], start=(kc == 0),
                                     stop=(kc == 1))
            ytn = chsb.tile([P, 2, P], F32, tag="yt")
            for dc in range(2):
                nc.vector.scalar_tensor_tensor(
                    ytn[:, dc, :], c3[:, dc, :], b3hap[dc], yt[:, dc, :],
                    ALU.add, ALU.add)
            yt = ytn

            ynp = chps.tile([P, D], F32, tag="ch")
            for dc in range(2):
                nc.tensor.transpose(ynp[:, dc * P:(dc + 1) * P], yt[:, dc, :], id32[:])
            yns = chsb.tile([P, D], F32, tag="yns")
            nc.vector.tensor_copy(yns[:], ynp[:])
            nc.sync.dma_start(out[:, k + 1, :], yns[:])

    outs = do_pair(NG - 1, NG - 2)
    chain(outs[NG - 1])
    for p in range(0, NG - 2, 2):
        ga = p
        gb = p + 1 if p + 1 < NG - 2 else None
        do_pair(ga, gb)
    nc.sync.dma_start(out[:, 0, :], lat[:, 0, :])


def _build(h):
    import concourse.mybir as mybir
    import concourse.tile as tile
    from concourse import bacc

    F32 = mybir.dt.float32
    BF16 = mybir.dt.bfloat16
    FP8 = mybir.dt.float8e4

    nc = bacc.Bacc("TRN2", target_bir_lowering=False, debug=False,
                   num_devices=NCORES)
    lat = nc.dram_tensor("lat", [PB, T_OBS, D], F32, kind="ExternalInput").ap()
    w3hd = nc.dram_tensor("w3h", [D, D], BF16, kind="ExternalInput").ap()
    w8d = nc.dram_tensor("w8", [3, D, D], FP8, kind="ExternalInput").ap()
    bpk = nc.dram_tensor("bpack", [P, 6], F32, kind="ExternalInput").ap()
    br8p = nc.dram_tensor("brows8", [1, 2 * D], FP8, kind="ExternalInput").ap()
    id8d = nc.dram_tensor("id8", [P, P], BF16, kind="ExternalInput").ap()
    id32d = nc.dram_tensor("id32", [P, P], F32, kind="ExternalInput").ap()
    ones8d = nc.dram_tensor("ones8", [1, P], FP8, kind="ExternalInput").ap()
    out = nc.dram_tensor("out", [PB, T, D], F32, kind="ExternalOutput").ap()

    with tile.TileContext(nc) as tc, ExitStack() as ctx:
        _emit(ctx, tc, lat, w3hd, w8d, bpk, br8p, id8d, id32d, ones8d, out, h)
    nc.compile()
    return nc


def _host_inputs(inputs):
    ts = np.asarray(inputs["time_steps"], np.float32)
    h = float(np.float32(ts[1]) - np.float32(ts[0]))

    bf = ml_dtypes.bfloat16
    f8 = ml_dtypes.float8_e4m3
    w3h = (np.asarray(inputs["W3"], np.float32) * np.float32(h)).astype(bf)
    b1 = np.asarray(inputs["b1"], np.float32)
    b2 = np.asarray(inputs["b2"], np.float32)
    b3h = np.asarray(inputs["b3"], np.float32) * np.float32(h)
    bpack = np.stack([b1[:P], b1[P:], b2[:P], b2[P:], b3h[:P], b3h[P:]],
                     axis=1).astype(np.float32)
    w8 = np.stack([
        (8.0 * np.asarray(inputs["W1"], np.float32)),
        (8.0 * np.asarray(inputs["W2"], np.float32)),
        (8.0 * np.asarray(inputs["W3"], np.float32)),
    ]).astype(f8)
    b3s8 = (8.0 * np.asarray(inputs["b3"], np.float32))
    brows8 = np.concatenate([b3s8, b3s8]).reshape(1, 2 * D).astype(f8)
    id8 = np.eye(P, dtype=np.float32).astype(bf)
    id32 = np.eye(P, dtype=np.float32)
    ones8 = np.ones((1, P), np.float32).astype(f8)

    shared = dict(w3h=w3h, w8=w8, bpack=bpack, brows8=brows8,
                  id8=id8, id32=id32, ones8=ones8)
    return h, shared


_CACHE = {}


def kernel(**inputs):
    from concourse.bass_utils import run_bass_kernel_spmd

    lat_full = np.ascontiguousarray(np.asarray(inputs["latents"], np.float32))
    h, shared = _host_inputs(inputs)

    if h not in _CACHE:
        _CACHE[h] = _build(h)
    nc = _CACHE[h]

    in_maps = []
    for c in range(NCORES):
        m = dict(shared)
        m["lat"] = np.ascontiguousarray(lat_full[c * PB:(c + 1) * PB])
        in_maps.append(m)
    res = run_bass_kernel_spmd(nc, in_maps, list(range(NCORES)))
    outs = [res.results[c]["out"] for c in range(NCORES)]
    return np.concatenate(outs, axis=0)



# revision 2
# speedup vs baseline: 1.2816x; 1.2816x over previous
"""Trainium2 Bass kernel for nn_ODE_71743133713072.

Semantics (unrolled from the reference lax.scan):
  out[:, 0]   = lat[:, 0]
  out[:, 2]   = lat[:, 1]                                (dt=0 scan quirk)
  out[:, t+1] = lat[:, t] + h * f(lat[:, t])   for t = 0, 2..99
  y = out[:, 100];  out[:, k+1] = y = y + h * f(y)  for k = 100..118
where f is the D->U->U->D tanh MLP and h = ts[1]-ts[0] (linspace; per-step
fp32 diffs differ from h by <=1 ulp, far below the fp8 matmul noise floor).

Everything on-device runs FEATURE-MAJOR ([d, batch*time]); the host
pre-transposes the inputs (free) and post-transposes the outputs (free),
so the PE does zero transposes and zero bias matmuls:
  - xT8:   fp8(lat^T)  -> L1 moving operand, DoubleRow K=256.
  - latTB: lat^T + h*b3 (f32) -> the Euler-add operand (b3 host-folded).
  - L1/L2/L3 all keep the (fp8, x8-scaled) weights stationary; activations
    are always the moving operand, so no role swap and N=512 per matmul.
  - Euler update is one fused stt: oT = mm3 * (h/8) + latTB.
Frames 0 and 2 are copied from lat on the host.  The 19-step prediction
chain stays feature-major too (state = oT[g=24] tail slice), bf16 L3 with
h pre-folded, and writes its frames to a separate buffer the host merges.
"""

import os
import sys
from contextlib import ExitStack

import numpy as np

for _p in ("/opt/trn_rl_repo", "/root/.axon_site/_ro/trn_rl_repo"):
    if os.path.isdir(_p) and _p not in sys.path:
        sys.path.append(_p)

import ml_dtypes  # noqa: E402

B, T_OBS, KPRED, D = 1024, 100, 20, 256
T = T_OBS + KPRED          # 120
NCORES = 8
PB = B // NCORES           # 128 rows per core
P = 128
G = 4                      # time steps per compute group
NG = T_OBS // G            # 25 groups
NCH = KPRED - 1            # 19 chain steps


def _emit(ctx, tc, latTB, xT8, w8d, w3hd, bpkd, outT, outR, h):
    import concourse.mybir as mybir

    nc = tc.nc
    F32 = mybir.dt.float32
    BF16 = mybir.dt.bfloat16
    FP8 = mybir.dt.float8e4
    AF = mybir.ActivationFunctionType
    ALU = mybir.AluOpType
    DR = mybir.MatmulPerfMode.DoubleRow

    const = ctx.enter_context(tc.tile_pool(name="const", bufs=1))
    # fp8 weights (x8-scaled), stationary layout [K_lo, ktile, M]
    w8sb = const.tile([P, 3, 2, D], FP8, tag="w8")
    for wi in range(3):
        for kc in range(2):
            nc.sync.dma_start(w8sb[:, wi, kc, :], w8d[wi, kc * P:(kc + 1) * P, :])
    # bf16 h*W3 for the chain's L3 (h folded; fp8 would underflow on h*W3)
    w3hsb = const.tile([P, 2, D], BF16, tag="w3h")
    for kc in range(2):
        nc.sync.dma_start(w3hsb[:, kc, :], w3hd[kc * P:(kc + 1) * P, :])
    bsb = const.tile([P, 6], F32, tag="bias")
    nc.sync.dma_start(bsb[:], bpkd[:])

    b1ap = [bsb[:, 0:1], bsb[:, 1:2]]
    b2ap = [bsb[:, 2:3], bsb[:, 3:4]]
    b3hap = [bsb[:, 4:5], bsb[:, 5:6]]

    latp = ctx.enter_context(tc.tile_pool(name="lat", bufs=3))
    x8p = ctx.enter_context(tc.tile_pool(name="x8", bufs=3))
    h1p = ctx.enter_context(tc.tile_pool(name="h1", bufs=2))
    h2p = ctx.enter_context(tc.tile_pool(name="h2", bufs=2))
    oTp = ctx.enter_context(tc.tile_pool(name="oT", bufs=3))
    ringp = ctx.enter_context(tc.tile_pool(name="ring", bufs=1))
    y8pool = ctx.enter_context(tc.tile_pool(name="y8", bufs=2))
    c1sp = ctx.enter_context(tc.tile_pool(name="c1s", bufs=2))
    c2sp = ctx.enter_context(tc.tile_pool(name="c2s", bufs=2))

    mmps = ctx.enter_context(tc.tile_pool(name="mmps", bufs=3, space="PSUM"))
    chps = ctx.enter_context(tc.tile_pool(name="chps", bufs=2, space="PSUM"))

    h8 = float(h / 8.0)

    def group(g):
        t0 = g * G
        xt = latp.tile([P, 2, G, P], F32, tag="lat")
        nc.sync.dma_start(xt[:], latTB[:, :, t0:t0 + G, :])
        x8 = x8p.tile([P, 2, G, P], FP8, tag="x8")
        nc.gpsimd.dma_start(x8[:], xT8[:, :, t0:t0 + G, :])
        x8f = x8.rearrange("p k t b -> p k (t b)")

        mm1 = mmps.tile([P, 2, G * P], F32, tag="mm", name="mm1")
        for mc in range(2):
            nc.tensor.matmul(mm1[:, mc, :],
                             w8sb[:, 0, :, mc * P:(mc + 1) * P],
                             x8f[:], start=True, stop=True, perf_mode=DR)
        h1 = h1p.tile([P, 2, G * P], FP8, tag="h1")
        for mc in range(2):
            nc.scalar.activation(h1[:, mc, :], mm1[:, mc, :], AF.Tanh,
                                 bias=b1ap[mc], scale=0.125)
        mm2 = mmps.tile([P, 2, G * P], F32, tag="mm", name="mm2")
        for mc in range(2):
            nc.tensor.matmul(mm2[:, mc, :],
                             w8sb[:, 1, :, mc * P:(mc + 1) * P],
                             h1[:], start=True, stop=True, perf_mode=DR)
        h2 = h2p.tile([P, 2, G * P], FP8, tag="h2")
        for mc in range(2):
            nc.scalar.activation(h2[:, mc, :], mm2[:, mc, :], AF.Tanh,
                                 bias=b2ap[mc], scale=0.125)
        mm3 = mmps.tile([P, 2, G * P], F32, tag="mm", name="mm3")
        for dc in range(2):
            nc.tensor.matmul(mm3[:, dc, :],
                             w8sb[:, 2, :, dc * P:(dc + 1) * P],
                             h2[:], start=True, stop=True, perf_mode=DR)
        oT = oTp.tile([P, 2, G * P], F32, tag="oT")
        nc.vector.scalar_tensor_tensor(
            oT.rearrange("p a b -> p (a b)"),
            mm3.rearrange("p a b -> p (a b)"), h8,
            xt.rearrange("p a t b -> p (a t b)"),
            ALU.mult, ALU.add)
        nc.sync.dma_start(outT[:, :, t0:t0 + G, :],
                          oT.rearrange("p a (t b) -> p a t b", t=G))
        return oT

    def chain(oT24):
        # y0 = out[:, 100]^T = last time slice of group 24's output
        ring = ringp.tile([P, NCH, 2, P], F32, tag="ring")
        ysrc = oT24[:, :, (G - 1) * P:G * P]
        y8 = y8pool.tile([P, 2, P], FP8, tag="y8")
        nc.vector.tensor_copy(y8[:], ysrc)

        for k in range(NCH):
            c1 = chps.tile([P, 2, P], F32, tag="ch", name="c1")
            for mc in range(2):
                nc.tensor.matmul(c1[:, mc, :],
                                 w8sb[:, 0, :, mc * P:(mc + 1) * P],
                                 y8[:], start=True, stop=True, perf_mode=DR)
            c1s = c1sp.tile([P, 2, P], FP8, tag="c1s")
            for mc in range(2):
                nc.scalar.activation(c1s[:, mc, :], c1[:, mc, :], AF.Tanh,
                                     bias=b1ap[mc], scale=0.125)
            c2 = chps.tile([P, 2, P], F32, tag="ch", name="c2")
            for mc in range(2):
                nc.tensor.matmul(c2[:, mc, :],
                                 w8sb[:, 1, :, mc * P:(mc + 1) * P],
                                 c1s[:], start=True, stop=True, perf_mode=DR)
            c2s = c2sp.tile([P, 2, P], BF16, tag="c2s")
            for mc in range(2):
                nc.scalar.activation(c2s[:, mc, :], c2[:, mc, :], AF.Tanh,
                                     bias=b2ap[mc], scale=0.125)
            # L3 in bf16 (w3hsb = h*W3); b3*h joins in the update stt below.
            c3 = chps.tile([P, 2, P], F32, tag="ch", name="c3")
            for dc in range(2):
                for kc in range(2):
                    nc.tensor.matmul(c3[:, dc, :],
                                     w3hsb[:, kc, dc * P:(dc + 1) * P],
                                     c2s[:, kc, :], start=(kc == 0),
                                     stop=(kc == 1))
            ynew = ring[:, k, :, :]
            if k < NCH - 1:
                y8n = y8pool.tile([P, 2, P], FP8, tag="y8")
                for dc in range(2):
                    nc.vector.scalar_tensor_tensor(
                        y8n[:, dc, :], c3[:, dc, :], b3hap[dc],
                        ysrc[:, dc, :], ALU.add, ALU.add)
                y8 = y8n
            for dc in range(2):
                nc.vector.scalar_tensor_tensor(
                    ynew[:, dc, :], c3[:, dc, :], b3hap[dc],
                    ysrc[:, dc, :], ALU.add, ALU.add)
            nc.gpsimd.dma_start(outR[:, k, :, :], ynew)
            ysrc = ynew

    oT24 = group(NG - 1)
    chain(oT24)
    for g in range(NG - 1):
        group(g)


def _build(h):
    import concourse.mybir as mybir
    import concourse.tile as tile
    from concourse import bacc

    F32 = mybir.dt.float32
    BF16 = mybir.dt.bfloat16
    FP8 = mybir.dt.float8e4

    nc = bacc.Bacc("TRN2", target_bir_lowering=False, debug=False,
                   num_devices=NCORES)
    latTB = nc.dram_tensor("latTB", [P, 2, T_OBS, PB], F32,
                           kind="ExternalInput").ap()
    xT8 = nc.dram_tensor("xT8", [P, 2, T_OBS, PB], FP8,
                         kind="ExternalInput").ap()
    w8d = nc.dram_tensor("w8", [3, D, D], FP8, kind="ExternalInput").ap()
    w3hd = nc.dram_tensor("w3h", [D, D], BF16, kind="ExternalInput").ap()
    bpkd = nc.dram_tensor("bpack", [P, 6], F32, kind="ExternalInput").ap()
    outT = nc.dram_tensor("outT", [P, 2, T_OBS, PB], F32,
                          kind="ExternalOutput").ap()
    outR = nc.dram_tensor("outR", [P, NCH, 2, PB], F32,
                          kind="ExternalOutput").ap()

    with tile.TileContext(nc) as tc, ExitStack() as ctx:
        _emit(ctx, tc, latTB, xT8, w8d, w3hd, bpkd, outT, outR, h)
    nc.compile()
    return nc


def _host_inputs(inputs):
    ts = np.asarray(inputs["time_steps"], np.float32)
    h = float(np.float32(ts[1]) - np.float32(ts[0]))

    bf = ml_dtypes.bfloat16
    f8 = ml_dtypes.float8_e4m3
    W1 = np.asarray(inputs["W1"], np.float32)
    W2 = np.asarray(inputs["W2"], np.float32)
    W3 = np.asarray(inputs["W3"], np.float32)
    b1 = np.asarray(inputs["b1"], np.float32)
    b2 = np.asarray(inputs["b2"], np.float32)
    b3 = np.asarray(inputs["b3"], np.float32)
    w8 = np.stack([8.0 * W1, 8.0 * W2, 8.0 * W3]).astype(f8)
    w3h = (W3 * np.float32(h)).astype(bf)
    b3h = (b3 * np.float32(h)).astype(np.float32)
    bpack = np.stack([b1[:P], b1[P:], b2[:P], b2[P:], b3h[:P], b3h[P:]],
                     axis=1).astype(np.float32)
    shared = dict(w8=w8, w3h=w3h, bpack=bpack)
    return h, shared, b3h


def _percore_inputs(lat_full, b3h):
    # lat_full [B, T_OBS, D] -> per-core latTB/xT8 [P, 2, T_OBS, PB]
    f8 = ml_dtypes.float8_e4m3
    x = lat_full.reshape(NCORES, PB, T_OBS, 2, P)   # [c, b, t, dc, p]
    xt = x.transpose(0, 4, 3, 2, 1)                 # [c, p, dc, t, b]
    b3r = b3h.reshape(2, P).transpose(1, 0)         # [p, dc]
    latTBs = (xt + b3r[None, :, :, None, None]).astype(np.float32)
    xT8s = np.ascontiguousarray(xt).astype(f8)
    return latTBs, xT8s


def _assemble(lat_full, results):
    out = np.empty((B, T, D), np.float32)
    for c in range(NCORES):
        sl = slice(c * PB, (c + 1) * PB)
        oT = results[c]["outT"]    # [P, 2, T_OBS, PB]
        oR = results[c]["outR"]    # [P, NCH, 2, PB]
        out[sl, 1:T_OBS + 1, :] = oT.transpose(3, 2, 1, 0).reshape(
            PB, T_OBS, D)
        out[sl, T_OBS + 1:, :] = oR.transpose(3, 1, 2, 0).reshape(PB, NCH, D)
    out[:, 0, :] = lat_full[:, 0, :]
    out[:, 2, :] = lat_full[:, 1, :]
    return out


_CACHE = {}


def make_in_maps(inputs):
    lat_full = np.ascontiguousarray(np.asarray(inputs["latents"], np.float32))
    h, shared, b3h = _host_inputs(inputs)
    if h not in _CACHE:
        _CACHE[h] = _build(h)
    nc = _CACHE[h]
    latTBs, xT8s = _percore_inputs(lat_full, b3h)
    in_maps = []
    for c in range(NCORES):
        m = dict(shared)
        m["latTB"] = np.ascontiguousarray(latTBs[c])
        m["xT8"] = xT8s[c]
        in_maps.append(m)
    return nc, in_maps, lat_full


def kernel(**inputs):
    from concourse.bass_utils import run_bass_kernel_spmd

    nc, in_maps, lat_full = make_in_maps(inputs)
    res = run_bass_kernel_spmd(nc, in_maps, list(range(NCORES)))
    return _assemble(lat_full, [res.results[c] for c in range(NCORES)])


# revision 4
# speedup vs baseline: 1.3698x; 1.0688x over previous
"""Trainium2 Bass kernel for nn_ODE_71743133713072.

Semantics (unrolled from the reference lax.scan):
  out[:, 0]   = lat[:, 0]
  out[:, 2]   = lat[:, 1]                                (dt=0 scan quirk)
  out[:, t+1] = lat[:, t] + h * f(lat[:, t])   for t = 0, 2..99
  y = out[:, 100];  out[:, k+1] = y = y + h * f(y)  for k = 100..118
where f is the D->U->U->D tanh MLP and h = ts[1]-ts[0] (linspace; per-step
fp32 diffs differ from h by <=1 ulp, far below the fp8 matmul noise floor).

Everything on-device runs FEATURE-MAJOR ([d, batch*time]); the host
pre-transposes the inputs and post-transposes the outputs, so the PE does
zero transposes and zero bias matmuls:
  - xT8:   fp8(lat^T)  -> L1 moving operand, DoubleRow K=256.
  - latTB: lat^T + h*b3 (f32) -> the Euler-add operand (b3 host-folded).
  - L1/L2/L3 keep the (fp8, x8-scaled) weights stationary; activations are
    always the moving operand, so no role swap and N=512 per matmul.
  - Euler update is one fused stt: oT = mm3 * (h/8) + latTB.
Frames 0 and 2 are copied from lat on the host.  The 19-step prediction
chain stays feature-major (state = oT[g=24] tail slice); its MLP biases
enter via DVE-seeded PSUM (matmul start=False accumulates on top), so each
act/stt is a single merged instruction and the serial path per step is
stt_fp8 -> 2 MM -> act -> 2 MM -> act -> 2 MM -> stt_fp8.

The chain is latency-bound while the stream is throughput-bound; in-order
engine queues head-of-line block if either is emitted in large runs.  The
emission therefore interleaves ONE chain step with ONE stream group at
matching sub-stage granularity (chain MMs just before group MMs, chain act
before group acts, ...), keeping parked chain ops within each engine's
wait-queue depth so ready stream work flows around them.
"""

import os
import sys
from contextlib import ExitStack

import numpy as np

for _p in ("/opt/trn_rl_repo", "/root/.axon_site/_ro/trn_rl_repo"):
    if os.path.isdir(_p) and _p not in sys.path:
        sys.path.append(_p)

import ml_dtypes  # noqa: E402

B, T_OBS, KPRED, D = 1024, 100, 20, 256
T = T_OBS + KPRED          # 120
NCORES = 8
PB = B // NCORES           # 128 rows per core
P = 128
G = 4                      # time steps per compute group
NG = T_OBS // G            # 25 groups
NCH = KPRED - 1            # 19 chain steps


def _emit(ctx, tc, latTB, xT8, w8d, bpkd, bseedd, outT, outR, h):
    import concourse.mybir as mybir

    nc = tc.nc
    F32 = mybir.dt.float32
    FP8 = mybir.dt.float8e4
    AF = mybir.ActivationFunctionType
    ALU = mybir.AluOpType
    DR = mybir.MatmulPerfMode.DoubleRow

    const = ctx.enter_context(tc.tile_pool(name="const", bufs=1))
    # fp8 weights (x8-scaled), stationary layout [K_lo, ktile, M]
    w8sb = const.tile([P, 3, 2, D], FP8, tag="w8")
    nc.sync.dma_start(w8sb[:], w8d.rearrange("w (k p) m -> p w k m", k=2))
    bsb = const.tile([P, 4], F32, tag="bias")
    nc.sync.dma_start(bsb[:], bpkd[:])
    # PSUM bias seeds for the chain: 8*b1 / 8*b2 / 8*b3, bcast along batch
    bseed = const.tile([P, 3, 2, P], F32, tag="bseed")
    nc.sync.dma_start(bseed[:], bseedd[:])

    b1ap = [bsb[:, 0:1], bsb[:, 1:2]]
    b2ap = [bsb[:, 2:3], bsb[:, 3:4]]

    latp = ctx.enter_context(tc.tile_pool(name="lat", bufs=4))
    x8p = ctx.enter_context(tc.tile_pool(name="x8", bufs=4))
    h1p = ctx.enter_context(tc.tile_pool(name="h1", bufs=2))
    h2p = ctx.enter_context(tc.tile_pool(name="h2", bufs=2))
    oTp = ctx.enter_context(tc.tile_pool(name="oT", bufs=3))
    ringp = ctx.enter_context(tc.tile_pool(name="ring", bufs=1))
    y8pool = ctx.enter_context(tc.tile_pool(name="y8", bufs=2))
    c1sp = ctx.enter_context(tc.tile_pool(name="c1s", bufs=2))
    c2sp = ctx.enter_context(tc.tile_pool(name="c2s", bufs=2))

    mmps = ctx.enter_context(tc.tile_pool(name="mmps", bufs=3, space="PSUM"))
    chps = ctx.enter_context(tc.tile_pool(name="chps", bufs=2, space="PSUM"))

    h8 = float(h / 8.0)

    # ---- stream group stages -------------------------------------------
    def g_load(g):
        t0 = g * G
        xt = latp.tile([P, 2, G, P], F32, tag="lat")
        nc.sync.dma_start(xt[:], latTB[:, :, t0:t0 + G, :])
        x8 = x8p.tile([P, 2, G, P], FP8, tag="x8")
        nc.gpsimd.dma_start(x8[:], xT8[:, :, t0:t0 + G, :])
        return dict(xt=xt, x8=x8)

    def g_mm(s, wi, key, src):
        mm = mmps.tile([P, 2, G * P], F32, tag="mm", name=f"mm{wi}")
        for mc in range(2):
            nc.tensor.matmul(mm[:, mc, :],
                             w8sb[:, wi, :, mc * P:(mc + 1) * P],
                             src[:], start=True, stop=True, perf_mode=DR)
        s[key] = mm

    def g_act(s, mmkey, key, pool, bap):
        ht = pool.tile([P, 2, G * P], FP8, tag=key)
        for mc in range(2):
            nc.scalar.activation(ht[:, mc, :], s[mmkey][:, mc, :], AF.Tanh,
                                 bias=bap[mc], scale=0.125)
        s[key] = ht

    def g_store(s, g):
        t0 = g * G
        oT = oTp.tile([P, 2, G * P], F32, tag="oT")
        nc.vector.scalar_tensor_tensor(
            oT.rearrange("p a b -> p (a b)"),
            s["mm3"].rearrange("p a b -> p (a b)"), h8,
            s["xt"].rearrange("p a t b -> p (a t b)"),
            ALU.mult, ALU.add)
        nc.sync.dma_start(outT[:, :, t0:t0 + G, :],
                          oT.rearrange("p a (t b) -> p a t b", t=G))
        s["oT"] = oT

    def g_all(s, g):
        g_mm(s, 0, "mm1", s["x8"].rearrange("p k t b -> p k (t b)"))
        g_act(s, "mm1", "h1", h1p, b1ap)
        g_mm(s, 1, "mm2", s["h1"])
        g_act(s, "mm2", "h2", h2p, b2ap)
        g_mm(s, 2, "mm3", s["h2"])
        g_store(s, g)

    # ---- chain stages ---------------------------------------------------
    def c_mm(wi, src, name):
        t = chps.tile([P, 2, P], F32, tag="ch", name=name)
        nc.vector.tensor_copy(t[:], bseed[:, wi, :, :])
        for mc in range(2):
            nc.tensor.matmul(t[:, mc, :],
                             w8sb[:, wi, :, mc * P:(mc + 1) * P],
                             src[:], start=False, stop=True, perf_mode=DR,
                             skip_group_check=True)
        return t

    def c_act(cin, pool, tag):
        t = pool.tile([P, 2, P], FP8, tag=tag)
        nc.scalar.activation(t[:], cin[:], AF.Tanh, scale=0.125)
        return t

    # ---- schedule -------------------------------------------------------
    s24 = g_load(NG - 1)
    g_all(s24, NG - 1)

    ring = ringp.tile([P, NCH, 2, P], F32, tag="ring")
    ysrc = s24["oT"][:, :, (G - 1) * P:G * P]
    y8 = y8pool.tile([P, 2, P], FP8, tag="y8")
    nc.vector.tensor_copy(y8[:], ysrc)

    order = list(range(NG - 1))
    states = {}
    states[order[0]] = g_load(order[0])
    states[order[1]] = g_load(order[1])
    for k in range(NCH):
        g = order[k]
        s = states[g]
        if k + 2 < len(order):
            states[order[k + 2]] = g_load(order[k + 2])
        c1 = c_mm(0, y8, "c1")
        g_mm(s, 0, "mm1", s["x8"].rearrange("p k t b -> p k (t b)"))
        c1s = c_act(c1, c1sp, "c1s")
        g_act(s, "mm1", "h1", h1p, b1ap)
        c2 = c_mm(1, c1s, "c2")
        g_mm(s, 1, "mm2", s["h1"])
        c2s = c_act(c2, c2sp, "c2s")
        g_act(s, "mm2", "h2", h2p, b2ap)
        c3 = c_mm(2, c2s, "c3")
        g_mm(s, 2, "mm3", s["h2"])
        ynew = ring[:, k, :, :]
        if k < NCH - 1:
            y8n = y8pool.tile([P, 2, P], FP8, tag="y8")
            nc.vector.scalar_tensor_tensor(y8n[:], c3[:], h8, ysrc,
                                           ALU.mult, ALU.add)
            y8 = y8n
        nc.vector.scalar_tensor_tensor(ynew, c3[:], h8, ysrc,
                                       ALU.mult, ALU.add)
        g_store(s, g)
        ysrc = ynew
        if k == 12:
            nc.gpsimd.dma_start(outR[:, 0:12, :, :], ring[:, 0:12, :, :])

    for g in order[NCH:]:
        s = states.get(g) or g_load(g)
        g_all(s, g)
    nc.gpsimd.dma_start(outR[:, 12:, :, :], ring[:, 12:, :, :])


def _build(h):
    import concourse.mybir as mybir
    import concourse.tile as tile
    from concourse import bacc

    F32 = mybir.dt.float32
    FP8 = mybir.dt.float8e4

    nc = bacc.Bacc("TRN2", target_bir_lowering=False, debug=False,
                   num_devices=NCORES)
    latTB = nc.dram_tensor("latTB", [P, 2, T_OBS, PB], F32,
                           kind="ExternalInput").ap()
    xT8 = nc.dram_tensor("xT8", [P, 2, T_OBS, PB], FP8,
                         kind="ExternalInput").ap()
    w8d = nc.dram_tensor("w8", [3, D, D], FP8, kind="ExternalInput").ap()
    bpkd = nc.dram_tensor("bpack", [P, 4], F32, kind="ExternalInput").ap()
    bseedd = nc.dram_tensor("bseed", [P, 3, 2, PB], F32,
                            kind="ExternalInput").ap()
    outT = nc.dram_tensor("outT", [P, 2, T_OBS, PB], F32,
                          kind="ExternalOutput").ap()
    outR = nc.dram_tensor("outR", [P, NCH, 2, PB], F32,
                          kind="ExternalOutput").ap()

    with tile.TileContext(nc) as tc, ExitStack() as ctx:
        _emit(ctx, tc, latTB, xT8, w8d, bpkd, bseedd, outT, outR, h)
    nc.compile()
    return nc


def _host_inputs(inputs):
    ts = np.asarray(inputs["time_steps"], np.float32)
    h = float(np.float32(ts[1]) - np.float32(ts[0]))

    f8 = ml_dtypes.float8_e4m3
    W1 = np.asarray(inputs["W1"], np.float32)
    W2 = np.asarray(inputs["W2"], np.float32)
    W3 = np.asarray(inputs["W3"], np.float32)
    b1 = np.asarray(inputs["b1"], np.float32)
    b2 = np.asarray(inputs["b2"], np.float32)
    b3 = np.asarray(inputs["b3"], np.float32)
    w8 = np.stack([8.0 * W1, 8.0 * W2, 8.0 * W3]).astype(f8)
    b3h = (b3 * np.float32(h)).astype(np.float32)
    bpack = np.stack([b1[:P], b1[P:], b2[:P], b2[P:]],
                     axis=1).astype(np.float32)
    # [P, 3, 2, PB]: 8*b{1,2,3}[mc*128+p] broadcast along batch
    bs = np.stack([8.0 * b1, 8.0 * b2, 8.0 * b3])        # [3, 256]
    bseed = np.ascontiguousarray(
        np.broadcast_to(bs.reshape(3, 2, P, 1).transpose(2, 0, 1, 3),
                        (P, 3, 2, PB))).astype(np.float32)
    shared = dict(w8=w8, bpack=bpack, bseed=bseed)
    return h, shared, b3h


def _percore_inputs(lat_full, b3h):
    # lat_full [B, T_OBS, D] -> per-core latTB/xT8 [P, 2, T_OBS, PB]
    f8 = ml_dtypes.float8_e4m3
    x = lat_full.reshape(NCORES, PB, T_OBS, 2, P)   # [c, b, t, dc, p]
    xt = x.transpose(0, 4, 3, 2, 1)                 # [c, p, dc, t, b]
    b3r = b3h.reshape(2, P).transpose(1, 0)         # [p, dc]
    latTBs = (xt + b3r[None, :, :, None, None]).astype(np.float32)
    xT8s = np.ascontiguousarray(xt).astype(f8)
    return latTBs, xT8s


def _assemble(lat_full, results):
    out = np.empty((B, T, D), np.float32)
    for c in range(NCORES):
        sl = slice(c * PB, (c + 1) * PB)
        oT = results[c]["outT"]    # [P, 2, T_OBS, PB]
        oR = results[c]["outR"]    # [P, NCH, 2, PB]
        out[sl, 1:T_OBS + 1, :] = oT.transpose(3, 2, 1, 0).reshape(
            PB, T_OBS, D)
        out[sl, T_OBS + 1:, :] = oR.transpose(3, 1, 2, 0).reshape(PB, NCH, D)
    out[:, 0, :] = lat_full[:, 0, :]
    out[:, 2, :] = lat_full[:, 1, :]
    return out


_CACHE = {}


def make_in_maps(inputs):
    lat_full = np.ascontiguousarray(np.asarray(inputs["latents"], np.float32))
    h, shared, b3h = _host_inputs(inputs)
    if h not in _CACHE:
        _CACHE[h] = _build(h)
    nc = _CACHE[h]
    latTBs, xT8s = _percore_inputs(lat_full, b3h)
    in_maps = []
    for c in range(NCORES):
        m = dict(shared)
        m["latTB"] = np.ascontiguousarray(latTBs[c])
        m["xT8"] = xT8s[c]
        in_maps.append(m)
    return nc, in_maps, lat_full


def kernel(**inputs):
    from concourse.bass_utils import run_bass_kernel_spmd

    nc, in_maps, lat_full = make_in_maps(inputs)
    res = run_bass_kernel_spmd(nc, in_maps, list(range(NCORES)))
    return _assemble(lat_full, [res.results[c] for c in range(NCORES)])


# revision 6
# speedup vs baseline: 1.5651x; 1.1425x over previous
"""Trainium2 Bass kernel for nn_ODE_71743133713072.

Semantics (unrolled from the reference lax.scan):
  out[:, 0]   = lat[:, 0]
  out[:, 2]   = lat[:, 1]                                (dt=0 scan quirk)
  out[:, t+1] = lat[:, t] + h * f(lat[:, t])   for t = 0, 2..99
  y = out[:, 100];  out[:, k+1] = y = y + h * f(y)  for k = 100..118
where f is the D->U->U->D tanh MLP and h = ts[1]-ts[0] (linspace; per-step
fp32 diffs differ from h by <=1 ulp, far below the fp8 matmul noise floor).

Everything on-device runs FEATURE-MAJOR ([d, batch*time]); the host
pre-transposes the inputs and post-transposes the outputs, so the PE does
zero transposes and zero bias matmuls:
  - xT8:   fp8(lat^T)  -> L1 moving operand, DoubleRow K=256.
  - latTB: lat^T + h*b3 (f32) -> the Euler-add operand (b3 host-folded).
  - L1/L2/L3 keep the (fp8, x8-scaled) weights stationary; activations are
    always the moving operand, so no role swap and N=512 per matmul.
  - Euler update is one fused stt: oT = mm3 * (h/8) + latTB.
Frames 0 and 2 are copied from lat on the host.  The 19-step prediction
chain stays feature-major (state = oT[g=24] tail slice); its MLP biases
enter via DVE-seeded PSUM (matmul start=False accumulates on top), so each
act/stt is a single merged instruction and the serial path per step is
stt_fp8 -> 2 MM -> act -> 2 MM -> act -> 2 MM -> stt_fp8.

The chain is latency-bound while the stream is throughput-bound; in-order
engine queues head-of-line block if either is emitted in large runs.  The
emission therefore interleaves ONE chain step with ONE stream group at
matching sub-stage granularity (chain MMs just before group MMs, chain act
before group acts, ...), keeping parked chain ops within each engine's
wait-queue depth so ready stream work flows around them.
"""

import os
import sys
from contextlib import ExitStack

import numpy as np

for _p in ("/opt/trn_rl_repo", "/root/.axon_site/_ro/trn_rl_repo"):
    if os.path.isdir(_p) and _p not in sys.path:
        sys.path.append(_p)

import ml_dtypes  # noqa: E402

B, T_OBS, KPRED, D = 1024, 100, 20, 256
T = T_OBS + KPRED          # 120
NCORES = 8
PB = B // NCORES           # 128 rows per core
P = 128
G = 4                      # time steps per compute group
NG = T_OBS // G            # 25 groups
NCH = KPRED - 1            # 19 chain steps


def _emit(ctx, tc, latTB, xT8, w8d, bpkd, bseedd, outT, outR, h):
    import concourse.mybir as mybir

    nc = tc.nc
    F32 = mybir.dt.float32
    FP8 = mybir.dt.float8e4
    AF = mybir.ActivationFunctionType
    ALU = mybir.AluOpType
    DR = mybir.MatmulPerfMode.DoubleRow

    const = ctx.enter_context(tc.tile_pool(name="const", bufs=1))
    # fp8 weights (x8-scaled), stationary layout [K_lo, ktile, M]
    w8sb = const.tile([P, 3, 2, D], FP8, tag="w8")
    nc.sync.dma_start(w8sb[:], w8d.rearrange("w (k p) m -> p w k m", k=2))
    bsb = const.tile([P, 4], F32, tag="bias")
    nc.sync.dma_start(bsb[:], bpkd[:])
    # PSUM bias seeds for the chain: 8*b1 / 8*b2 / 8*b3, bcast along batch
    bseed = const.tile([P, 3, 2, P], F32, tag="bseed")
    nc.sync.dma_start(bseed[:], bseedd[:])

    b1ap = [bsb[:, 0:1], bsb[:, 1:2]]
    b2ap = [bsb[:, 2:3], bsb[:, 3:4]]

    latp = ctx.enter_context(tc.tile_pool(name="lat", bufs=4))
    x8p = ctx.enter_context(tc.tile_pool(name="x8", bufs=4))
    h1p = ctx.enter_context(tc.tile_pool(name="h1", bufs=3))
    h2p = ctx.enter_context(tc.tile_pool(name="h2", bufs=3))
    oTp = ctx.enter_context(tc.tile_pool(name="oT", bufs=3))
    ringp = ctx.enter_context(tc.tile_pool(name="ring", bufs=1))
    y8pool = ctx.enter_context(tc.tile_pool(name="y8", bufs=2))
    c1sp = ctx.enter_context(tc.tile_pool(name="c1s", bufs=2))
    c2sp = ctx.enter_context(tc.tile_pool(name="c2s", bufs=2))
    minip = ctx.enter_context(tc.tile_pool(name="mini", bufs=1))

    # one PSUM bank per tile: fine-grained recycling so a stream matmul never
    # parks long at the PE queue head waiting for an act to free a bank
    mmps = ctx.enter_context(tc.tile_pool(name="mmps", bufs=6, space="PSUM"))
    chps = ctx.enter_context(tc.tile_pool(name="chps", bufs=2, space="PSUM"))

    h8 = float(h / 8.0)

    # ---- stream group stages -------------------------------------------
    def g_load(g):
        t0 = g * G
        xt = latp.tile([P, 2, G, P], F32, tag="lat")
        nc.sync.dma_start(xt[:], latTB[:, :, t0:t0 + G, :])
        x8 = x8p.tile([P, 2, G, P], FP8, tag="x8")
        nc.gpsimd.dma_start(x8[:], xT8[:, :, t0:t0 + G, :])
        return dict(xt=xt, x8=x8)

    def g_mm(s, wi, key, src):
        mm = []
        for mc in range(2):
            t = mmps.tile([P, G * P], F32, tag="mm", name=f"mm{wi}_{mc}")
            nc.tensor.matmul(t[:],
                             w8sb[:, wi, :, mc * P:(mc + 1) * P],
                             src[:], start=True, stop=True, perf_mode=DR)
            mm.append(t)
        s[key] = mm

    def g_act(s, mmkey, key, pool, bap):
        ht = pool.tile([P, 2, G * P], FP8, tag=key)
        for mc in range(2):
            nc.scalar.activation(ht[:, mc, :], s[mmkey][mc][:], AF.Tanh,
                                 bias=bap[mc], scale=0.125)
        s[key] = ht

    def g_store(s, g):
        t0 = g * G
        oT = oTp.tile([P, 2, G * P], F32, tag="oT")
        for dc in range(2):
            nc.vector.scalar_tensor_tensor(
                oT[:, dc, :], s["mm3"][dc][:], h8,
                s["xt"].rearrange("p a t b -> p a (t b)")[:, dc, :],
                ALU.mult, ALU.add)
        nc.sync.dma_start(outT[:, :, t0:t0 + G, :],
                          oT.rearrange("p a (t b) -> p a t b", t=G))
        s["oT"] = oT

    def g_all(s, g):
        g_mm(s, 0, "mm1", s["x8"].rearrange("p k t b -> p k (t b)"))
        g_act(s, "mm1", "h1", h1p, b1ap)
        g_mm(s, 1, "mm2", s["h1"])
        g_act(s, "mm2", "h2", h2p, b2ap)
        g_mm(s, 2, "mm3", s["h2"])
        g_store(s, g)

    # ---- chain stages ---------------------------------------------------
    def c_mm(wi, src, name):
        t = chps.tile([P, 2, P], F32, tag="ch", name=name)
        nc.vector.tensor_copy(t[:], bseed[:, wi, :, :])
        for mc in range(2):
            nc.tensor.matmul(t[:, mc, :],
                             w8sb[:, wi, :, mc * P:(mc + 1) * P],
                             src[:], start=False, stop=True, perf_mode=DR,
                             skip_group_check=True)
        return t

    def c_act(cin, pool, tag):
        t = pool.tile([P, 2, P], FP8, tag=tag)
        nc.scalar.activation(t[:], cin[:], AF.Tanh, scale=0.125)
        return t

    # ---- schedule -------------------------------------------------------
    # mini pre-group: compute y0 = out[:,100] from t=99 alone so the chain
    # starts ~6us in, instead of waiting for all of group 24
    xt99 = minip.tile([P, 2, 1, P], F32, tag="xt99")
    nc.sync.dma_start(xt99[:], latTB[:, :, T_OBS - 1:T_OBS, :])
    x899 = minip.tile([P, 2, 1, P], FP8, tag="x899")
    nc.gpsimd.dma_start(x899[:], xT8[:, :, T_OBS - 1:T_OBS, :])
    m1 = c_mm(0, x899.rearrange("p a t b -> p a (t b)"), "m1")
    m1s = c_act(m1, c1sp, "c1s")
    m2 = c_mm(1, m1s, "m2")
    m2s = c_act(m2, c2sp, "c2s")
    m3 = chps.tile([P, 2, P], F32, tag="ch", name="m3")
    for mc in range(2):
        nc.tensor.matmul(m3[:, mc, :], w8sb[:, 2, :, mc * P:(mc + 1) * P],
                         m2s[:], start=True, stop=True, perf_mode=DR)
    y0 = minip.tile([P, 2, P], F32, tag="y0")
    nc.vector.scalar_tensor_tensor(y0[:], m3[:], h8,
                                   xt99.rearrange("p a t b -> p a (t b)"),
                                   ALU.mult, ALU.add)

    ring = ringp.tile([P, NCH, 2, P], F32, tag="ring")
    ysrc = y0[:, :, :]
    y8 = y8pool.tile([P, 2, P], FP8, tag="y8")
    nc.vector.tensor_copy(y8[:], ysrc)

    order = [NG - 1] + list(range(NG - 1))
    states = {}
    states[order[0]] = g_load(order[0])
    states[order[1]] = g_load(order[1])
    for k in range(NCH):
        g = order[k]
        s = states[g]
        if k + 2 < len(order):
            states[order[k + 2]] = g_load(order[k + 2])
        c1 = c_mm(0, y8, "c1")
        g_mm(s, 0, "mm1", s["x8"].rearrange("p k t b -> p k (t b)"))
        c1s = c_act(c1, c1sp, "c1s")
        g_act(s, "mm1", "h1", h1p, b1ap)
        c2 = c_mm(1, c1s, "c2")
        g_mm(s, 1, "mm2", s["h1"])
        c2s = c_act(c2, c2sp, "c2s")
        g_act(s, "mm2", "h2", h2p, b2ap)
        c3 = c_mm(2, c2s, "c3")
        g_mm(s, 2, "mm3", s["h2"])
        ynew = ring[:, k, :, :]
        if k < NCH - 1:
            y8n = y8pool.tile([P, 2, P], FP8, tag="y8")
            nc.vector.scalar_tensor_tensor(y8n[:], c3[:], h8, ysrc,
                                           ALU.mult, ALU.add)
            y8 = y8n
        nc.vector.scalar_tensor_tensor(ynew, c3[:], h8, ysrc,
                                       ALU.mult, ALU.add)
        g_store(s, g)
        ysrc = ynew
        if k == 12:
            nc.gpsimd.dma_start(outR[:, 0:12, :, :], ring[:, 0:12, :, :])

    for g in order[NCH:]:
        s = states.get(g) or g_load(g)
        g_all(s, g)
    nc.gpsimd.dma_start(outR[:, 12:, :, :], ring[:, 12:, :, :])


def _build(h):
    import concourse.mybir as mybir
    import concourse.tile as tile
    from concourse import bacc

    F32 = mybir.dt.float32
    FP8 = mybir.dt.float8e4

    nc = bacc.Bacc("TRN2", target_bir_lowering=False, debug=False,
                   num_devices=NCORES)
    latTB = nc.dram_tensor("latTB", [P, 2, T_OBS, PB], F32,
                           kind="ExternalInput").ap()
    xT8 = nc.dram_tensor("xT8", [P, 2, T_OBS, PB], FP8,
                         kind="ExternalInput").ap()
    w8d = nc.dram_tensor("w8", [3, D, D], FP8, kind="ExternalInput").ap()
    bpkd = nc.dram_tensor("bpack", [P, 4], F32, kind="ExternalInput").ap()
    bseedd = nc.dram_tensor("bseed", [P, 3, 2, PB], F32,
                            kind="ExternalInput").ap()
    outT = nc.dram_tensor("outT", [P, 2, T_OBS, PB], F32,
                          kind="ExternalOutput").ap()
    outR = nc.dram_tensor("outR", [P, NCH, 2, PB], F32,
                          kind="ExternalOutput").ap()

    with tile.TileContext(nc) as tc, ExitStack() as ctx:
        _emit(ctx, tc, latTB, xT8, w8d, bpkd, bseedd, outT, outR, h)
    nc.compile()
    return nc


def _host_inputs(inputs):
    ts = np.asarray(inputs["time_steps"], np.float32)
    h = float(np.float32(ts[1]) - np.float32(ts[0]))

    f8 = ml_dtypes.float8_e4m3
    W1 = np.asarray(inputs["W1"], np.float32)
    W2 = np.asarray(inputs["W2"], np.float32)
    W3 = np.asarray(inputs["W3"], np.float32)
    b1 = np.asarray(inputs["b1"], np.float32)
    b2 = np.asarray(inputs["b2"], np.float32)
    b3 = np.asarray(inputs["b3"], np.float32)
    w8 = np.stack([8.0 * W1, 8.0 * W2, 8.0 * W3]).astype(f8)
    b3h = (b3 * np.float32(h)).astype(np.float32)
    bpack = np.stack([b1[:P], b1[P:], b2[:P], b2[P:]],
                     axis=1).astype(np.float32)
    # [P, 3, 2, PB]: 8*b{1,2,3}[mc*128+p] broadcast along batch
    bs = np.stack([8.0 * b1, 8.0 * b2, 8.0 * b3])        # [3, 256]
    bseed = np.ascontiguousarray(
        np.broadcast_to(bs.reshape(3, 2, P, 1).transpose(2, 0, 1, 3),
                        (P, 3, 2, PB))).astype(np.float32)
    shared = dict(w8=w8, bpack=bpack, bseed=bseed)
    return h, shared, b3h


def _percore_inputs(lat_full, b3h):
    # lat_full [B, T_OBS, D] -> per-core latTB/xT8 [P, 2, T_OBS, PB]
    f8 = ml_dtypes.float8_e4m3
    x = lat_full.reshape(NCORES, PB, T_OBS, 2, P)   # [c, b, t, dc, p]
    xt = x.transpose(0, 4, 3, 2, 1)                 # [c, p, dc, t, b]
    b3r = b3h.reshape(2, P).transpose(1, 0)         # [p, dc]
    latTBs = (xt + b3r[None, :, :, None, None]).astype(np.float32)
    xT8s = np.ascontiguousarray(xt).astype(f8)
    return latTBs, xT8s


def _assemble(lat_full, results):
    out = np.empty((B, T, D), np.float32)
    for c in range(NCORES):
        sl = slice(c * PB, (c + 1) * PB)
        oT = results[c]["outT"]    # [P, 2, T_OBS, PB]
        oR = results[c]["outR"]    # [P, NCH, 2, PB]
        out[sl, 1:T_OBS + 1, :] = oT.transpose(3, 2, 1, 0).reshape(
            PB, T_OBS, D)
        out[sl, T_OBS + 1:, :] = oR.transpose(3, 1, 2, 0).reshape(PB, NCH, D)
    out[:, 0, :] = lat_full[:, 0, :]
    out[:, 2, :] = lat_full[:, 1, :]
    return out


_CACHE = {}


def make_in_maps(inputs):
    lat_full = np.ascontiguousarray(np.asarray(inputs["latents"], np.float32))
    h, shared, b3h = _host_inputs(inputs)
    if h not in _CACHE:
        _CACHE[h] = _build(h)
    nc = _CACHE[h]
    latTBs, xT8s = _percore_inputs(lat_full, b3h)
    in_maps = []
    for c in range(NCORES):
        m = dict(shared)
        m["latTB"] = np.ascontiguousarray(latTBs[c])
        m["xT8"] = xT8s[c]
        in_maps.append(m)
    return nc, in_maps, lat_full


def kernel(**inputs):
    from concourse.bass_utils import run_bass_kernel_spmd

    nc, in_maps, lat_full = make_in_maps(inputs)
    res = run_bass_kernel_spmd(nc, in_maps, list(range(NCORES)))
    return _assemble(lat_full, [res.results[c] for c in range(NCORES)])
